# revision 4
# baseline (speedup 1.0000x reference)
"""Dense MoE (all-experts, gate-weighted sum) on 8 Trainium2 NeuronCores.

Sharding: pure data-parallel over the token axis N (8192 -> 1024 rows/core);
every core holds all 8 experts, so no collectives are needed.

Mean-centered fp8 decomposition (the key trick):
    out = x @ Wbar.T  +  sum_e (g_e - 1/8) * (x @ W_e.T)  +  gates @ b_e
with Wbar = mean_e W_e. The bulk term x@Wbar runs in bf16; the 8 expert
GEMMs run as e4m3 DoubleRow matmuls (K=256/instruction -> 2x PE through-
put, measured 216 ns per FD=512 DR matmul, same as bf16). Because the
correction weights delta_e = g_e - 1/8 have std ~0.03, the fp8
quantization noise of the expert GEMMs is attenuated ~30x in the output:
simulated end-to-end rel err ~1.1e-2 (pure-fp8 would be 3.3e-2).

Per core (N_loc=1024, D=1024, E=8, O=1024, H=256) the PE busy time is
    gating 6.9us + Wbar(bf16) 27.6us + experts(fp8 DR) 110.6us + misc
vs 218us+ for all-bf16 experts.

Schedule skeleton is inherited from the bf16 baseline: host-prearranged
layouts (partition-major, large DMA descriptors); warmup+filler matmuls
hold the PE HAM clock at 2.4 GHz across the initial DMA; the gating GEMM
and seven Wbar "wave-1" PSUM groups accumulate dk-progressively as x/Wbar
quads land on the sync ring; logits for all 8 n-tiles collect in one PSUM
bank -> single batched softmax (exp preloaded, no max-subtract); gates.T
via 2 zero-padded PE transposes feeds 16 K=8 bias matmuls; expert drains
are one fused DVE op acc = (psum * delta) + acc. The dequant scale
1/(SX*SW) is folded into delta on device. The last expert drains into
bf16 tiles (host upcasts to fp32), halving output DMA bytes; its final
tile drains in 256-wide chunks with DMAs alternating across issue queues.
"""

import numpy as np
import ml_dtypes

import concourse.bass as bass
import concourse.mybir as mybir
import concourse.tile as tile
from concourse.bass_utils import run_bass_kernel_spmd

N, D, E, O, H = 8192, 1024, 8, 1024, 256
NCORES = 8
NLOC = N // NCORES          # 1024 rows per core
P = 128                     # partitions
NT = NLOC // P              # 8 n-tiles
DK = D // P                 # 8 contraction tiles (bf16 path)
DK2 = D // 256              # 4 double-row contraction tiles (fp8 path)
FO = 512                    # matmul moving free dim (one PSUM bank of fp32)
OH = O // FO                # 2 output halves
H2 = H // P                 # 2 h-tiles
BF16 = mybir.dt.bfloat16
F8E4 = mybir.dt.float8e4
F32 = mybir.dt.float32
BF = ml_dtypes.bfloat16
F8 = ml_dtypes.float8_e4m3fn
DR = mybir.MatmulPerfMode.DoubleRow
N_WARM = 8                  # pre-stream HAM warmup matmuls
N_FILL = 14                 # filler matmuls bridging the DMA-paced phase
W1A = 4                     # wave-1a: Wbar (oh0, nt<4) groups in mm pool
W1B = 3                     # wave-1b: Wbar (oh1, nt<3) groups in g pool
SX = 32.0                   # host scale for x -> e4m3
SW = 2048.0                 # host scale for W_e -> e4m3
DEQ = 1.0 / (SX * SW)       # folded into delta on device


def legalize_single_wait(nc, max_waits=1):
    """This walrus build rejects instructions carrying more than one sync
    wait. Split each multi-wait instruction: excess waits move onto fresh
    same-engine NoOps inserted immediately before it (identical semantics:
    the engine stalls at the same program point on every semaphore)."""
    for f in nc.m.functions:
        for blk in f.blocks:
            insts = list(blk.instructions)
            if all(
                (i.sync_info is None or len(i.sync_info.on_wait) <= max_waits)
                for i in insts
            ):
                continue
            new = []
            for inst in insts:
                si = inst.sync_info
                if si is not None and len(si.on_wait) > max_waits:
                    waits = list(si.on_wait)
                    for k, w in enumerate(waits[:-max_waits]):
                        nop = mybir.InstNoOp(name=f"{inst.name}-w{k}")
                        nop.engine = inst.engine
                        nop.sync_info = mybir.SyncInfo(on_wait=[w], on_update=[])
                        new.append(nop)
                    si.on_wait = waits[-max_waits:]
                new.append(inst)
            blk.instructions = new
    return nc


def build_moe():
    nc = bass.Bass(target_bir_lowering=False)
    xT = nc.dram_tensor("xT", [P, DK, NLOC], BF16, kind="ExternalInput")
    xq = nc.dram_tensor("xq", [P, DK2, 2, NLOC], F8E4, kind="ExternalInput")
    wbar = nc.dram_tensor("wbar", [P, DK, O], BF16, kind="ExternalInput")
    wq = nc.dram_tensor("wq", [E, P, DK2, 2, O], F8E4, kind="ExternalInput")
    wg1t = nc.dram_tensor("wg1t", [P, DK, H], BF16, kind="ExternalInput")
    wg2t = nc.dram_tensor("wg2t", [P, H2, E], BF16, kind="ExternalInput")
    bg1 = nc.dram_tensor("bg1", [P, H2], F32, kind="ExternalInput")
    bg2 = nc.dram_tensor("bg2", [E], BF16, kind="ExternalInput")
    be_rep = nc.dram_tensor("be_rep", [P, O], BF16, kind="ExternalInput")
    ident = nc.dram_tensor("ident", [P, P], F32, kind="ExternalInput")
    out = nc.dram_tensor("out", [NT, OH, P, FO], BF16, kind="ExternalOutput")

    with tile.TileContext(nc) as tc:
        with (
            tc.tile_pool(name="const", bufs=1) as constp,
            tc.tile_pool(name="wpool", bufs=2) as wpool,
            tc.tile_pool(name="work", bufs=4) as workp,
            tc.tile_pool(name="g_ps", bufs=4, space="PSUM") as gp,
            tc.tile_pool(name="mm_ps", bufs=4, space="PSUM") as mmp,
        ):
            # ---- PE warm-up + ACT Exp-table preload during the DMA wait ----
            warm_a = constp.tile([P, P], BF16, tag="warm_a")
            nc.gpsimd.memset(warm_a, 0.0)
            warm_b = constp.tile([P, FO], BF16, tag="warm_b")
            nc.gpsimd.memset(warm_b, 0.0)
            for i in range(N_WARM):
                wpsum = mmp.tile([P, FO], F32, tag="mm", name=f"warm{i}")
                nc.tensor.matmul(wpsum, warm_a, warm_b, start=True, stop=True)
            dummy_exp = workp.tile([1, 1], F32, tag="dummy")
            nc.scalar.activation(
                out=dummy_exp,
                in_=warm_b[0:1, 0:1],
                func=mybir.ActivationFunctionType.Exp,
            )

            # ---- resident inputs. sync ring (FIFO = priority): x halves
            # interleaved with Wbar halves, then xq, then fp8 experts.
            # gpsimd ring: gating weights + small consts ----
            wg1t_sb = constp.tile([P, DK, H], BF16, tag="wg1t")
            xT_sb = constp.tile([P, DK, NLOC], BF16, tag="xT")
            wbar_sb = constp.tile([P, DK, O], BF16, tag="wbar")
            xq_sb = constp.tile([P, DK2, 2, NLOC], F8E4, tag="xq")
            nc.gpsimd.dma_start(out=wg1t_sb, in_=wg1t[:, :, :])
            # first x quad split in two so gating matmuls start ~2us earlier
            nc.sync.dma_start(out=xT_sb[:, 0:2, :], in_=xT[:, 0:2, :])
            nc.sync.dma_start(out=xT_sb[:, 2:4, :], in_=xT[:, 2:4, :])
            nc.sync.dma_start(out=wbar_sb[:, 0:4, :], in_=wbar[:, 0:4, :])
            nc.sync.dma_start(out=xT_sb[:, 4:8, :], in_=xT[:, 4:8, :])
            nc.sync.dma_start(out=wbar_sb[:, 4:8, :], in_=wbar[:, 4:8, :])
            nc.sync.dma_start(out=xq_sb, in_=xq[:, :, :, :])

            wg2t_sb = constp.tile([P, H2, E], BF16, tag="wg2t")
            nc.gpsimd.dma_start(out=wg2t_sb, in_=wg2t[:, :, :])
            bg1_sb = constp.tile([P, H2], F32, tag="bg1")
            nc.gpsimd.dma_start(out=bg1_sb, in_=bg1[:, :])
            bg2_sb = constp.tile([1, E], BF16, tag="bg2")
            nc.gpsimd.dma_start(out=bg2_sb, in_=bg2[:])
            ones_sb = constp.tile([1, P], BF16, tag="ones")
            nc.gpsimd.memset(ones_sb, 1.0)
            # ident/be_rep are needed only from bias-matmul time: they ride
            # the sync ring AFTER xq (FIFO ring = priority)
            ident_sb = constp.tile([P, P], F32, tag="ident")
            nc.sync.dma_start(out=ident_sb, in_=ident[:, :])
            be_sb = constp.tile([P, O], BF16, tag="be_rep")
            nc.sync.dma_start(out=be_sb, in_=be_rep[:, :])

            # ---- gating GEMM (4 psum banks) + wave-1 Wbar groups,
            # accumulating dk-progressively as halves land ----
            hT_sb = [
                constp.tile([P, NLOC], BF16, tag=f"hT{h2}", name=f"hT{h2}")
                for h2 in range(H2)
            ]
            psum_g = {
                (h2, nh): gp.tile([P, FO], F32, tag="g", name=f"psum_g{h2}_{nh}")
                for h2 in range(H2)
                for nh in range(NLOC // FO)
            }

            def gating_mms(dks):
                for dk in dks:
                    for h2 in range(H2):
                        for nh in range(NLOC // FO):
                            nc.tensor.matmul(
                                psum_g[(h2, nh)],
                                wg1t_sb[:, dk, h2 * P : (h2 + 1) * P],
                                xT_sb[:, dk, nh * FO : (nh + 1) * FO],
                                start=(dk == 0),
                                stop=(dk == DK - 1),
                            )

            # fillers: bridge the PE from the warmup to the first xT quad
            # landing (DMA-bound) so the HAM clock never re-throttles
            for i in range(N_FILL):
                wpsum = mmp.tile([P, FO], F32, tag="mm", name=f"fill{i}")
                nc.tensor.matmul(wpsum, warm_a, warm_b, start=True, stop=True)

            gating_mms(range(0, 4))

            # wave-1a: four Wbar (oh0) groups accumulate dk 0-3 as soon as
            # the first wbar quad lands (mm-pool banks)
            psum_w1a = [
                mmp.tile([P, FO], F32, tag="mm", name=f"w1a{i}") for i in range(W1A)
            ]

            def wave1a_mms(dks):
                for i in range(W1A):         # (nt=i, oh=0)
                    for dk in dks:
                        nc.tensor.matmul(
                            psum_w1a[i],
                            xT_sb[:, dk, i * P : (i + 1) * P],
                            wbar_sb[:, dk, 0:FO],
                            start=(dk == 0),
                            stop=(dk == DK - 1),
                        )

            wave1a_mms(range(0, 4))
            gating_mms(range(4, 8))

            # relus on DVE, ordered so logits for nt<4 unblock after 2 ops
            for nh in range(NLOC // FO):
                for h2 in range(H2):
                    nc.vector.tensor_scalar(
                        out=hT_sb[h2][:, nh * FO : (nh + 1) * FO],
                        in0=psum_g[(h2, nh)],
                        scalar1=bg1_sb[:, h2 : h2 + 1],
                        scalar2=0.0,
                        op0=mybir.AluOpType.add,
                        op1=mybir.AluOpType.max,
                    )

            # wave-1b + logits recycle the gating pool's banks (freed by the
            # relus above)
            psum_w1b = [
                gp.tile([P, FO], F32, tag="g", name=f"w1b{i}") for i in range(W1B)
            ]
            psum_l = gp.tile([P, NT, E], F32, tag="g", name="psum_l")

            def wave1b_mms(dks):
                for i in range(W1B):         # (nt=i, oh=1)
                    for dk in dks:
                        nc.tensor.matmul(
                            psum_w1b[i],
                            xT_sb[:, dk, i * P : (i + 1) * P],
                            wbar_sb[:, dk, FO : 2 * FO],
                            start=(dk == 0),
                            stop=(dk == DK - 1),
                        )

            wave1b_mms(range(0, 4))

            # logits for all n-tiles in ONE psum bank
            for nt in range(NT):
                for h2 in range(H2):
                    nc.tensor.matmul(
                        psum_l[:, nt, :],
                        hT_sb[h2][:, nt * P : (nt + 1) * P],
                        wg2t_sb[:, h2, :],
                        start=(h2 == 0),
                        stop=False,
                    )
                nc.tensor.matmul(
                    psum_l[:, nt, :], ones_sb, bg2_sb, start=False, stop=True
                )

            wave1a_mms(range(4, 8))
            wave1b_mms(range(4, 8))

            # batched softmax (no max-subtract: logits are O(1)); gates
            # zero-padded to 32 per n-tile so the transposed layout is
            # 32-row aligned (tile_position row groups for the bias matmuls)
            EP = 32
            gates_g = [
                constp.tile([P, NT // 2, EP], F32, tag=f"gates{g}", name=f"gates{g}")
                for g in range(2)
            ]
            for g in range(2):
                nc.vector.memset(gates_g[g], 0.0)
            gates_at = lambda nt: gates_g[nt // 4][:, nt % 4, 0:E]
            for g in range(2):
                nc.scalar.activation(
                    out=gates_g[g][:, :, 0:E],
                    in_=psum_l[:, 4 * g : 4 * (g + 1), :],
                    func=mybir.ActivationFunctionType.Exp,
                )
            sumexp = workp.tile([P, NT, 1], F32, tag="sumexp")
            for g in range(2):
                nc.vector.reduce_sum(
                    sumexp[:, 4 * g : 4 * (g + 1), :],
                    gates_g[g][:, :, 0:E],
                    axis=mybir.AxisListType.X,
                )
            rsum = workp.tile([P, NT, 1], F32, tag="rsum")
            nc.vector.reciprocal(rsum, sumexp)
            for nt in range(NT):
                nc.vector.tensor_scalar_mul(
                    gates_at(nt), gates_at(nt), rsum[:, nt, :]
                )

            # delta = (gates - 1/8) * DEQ  (dequant scale folded in);
            # padding columns get garbage but are never read
            delta_g = [
                constp.tile([P, NT // 2, EP], F32, tag=f"delta{g}", name=f"delta{g}")
                for g in range(2)
            ]
            for g in range(2):
                nc.vector.tensor_scalar(
                    out=delta_g[g],
                    in0=gates_g[g],
                    scalar1=-1.0 / E,
                    scalar2=DEQ,
                    op0=mybir.AluOpType.add,
                    op1=mybir.AluOpType.mult,
                )
            delta_at = lambda nt: delta_g[nt // 4][:, nt % 4, 0:E]

            acc_sb = [
                [
                    constp.tile(
                        [P, FO], F32, tag=f"acc{nt}_{oh}", name=f"acc{nt}_{oh}"
                    )
                    for oh in range(OH)
                ]
                for nt in range(NT)
            ]
            # bf16 staging tiles for the output of the last expert
            out_sb = [
                [
                    constp.tile(
                        [P, FO], BF16, tag=f"out{nt}_{oh}", name=f"out{nt}_{oh}"
                    )
                    for oh in range(OH)
                ]
                for nt in range(NT)
            ]

            # wave-1 drains: acc = psum (the Wbar term initializes acc)
            for i in range(W1A):
                nc.scalar.copy(out=acc_sb[i][0], in_=psum_w1a[i])
            for i in range(W1B):
                nc.scalar.copy(out=acc_sb[i][1], in_=psum_w1b[i])

            # gates.T via two 128x128 PE transposes (rows nt*32+e, 32-aligned),
            # hidden inside the Wbar matmul stream
            gatesT = []
            for g in range(2):
                psum_t = gp.tile([P, P], F32, tag="g", name="psum_t")
                nc.tensor.transpose(
                    psum_t, gates_g[g].rearrange("p a b -> p (a b)"), ident_sb
                )
                gT = constp.tile([P, P], BF16, tag=f"gatesT{g}", name=f"gatesT{g}")
                nc.scalar.copy(out=gT, in_=psum_t)
                gatesT.append(gT)

            # ---- remaining 9 Wbar tiles (closed groups, mm pool) ----
            def wbar_tile(nt, oh, pool, tg):
                psum = pool.tile([P, FO], F32, tag=tg, name=f"wb{nt}_{oh}")
                for dk in range(DK):
                    nc.tensor.matmul(
                        psum,
                        xT_sb[:, dk, nt * P : (nt + 1) * P],
                        wbar_sb[:, dk, oh * FO : (oh + 1) * FO],
                        start=(dk == 0),
                        stop=(dk == DK - 1),
                    )
                nc.scalar.copy(out=acc_sb[nt][oh], in_=psum)

            rest = [(nt, 0) for nt in range(W1A, NT)] + [
                (nt, 1) for nt in range(W1B, NT)
            ]
            for j, (nt, oh) in enumerate(rest):
                pool, tg = ((mmp, "mm"), (gp, "g"))[j % 2]
                wbar_tile(nt, oh, pool, tg)

            # bias term: out += gates @ b_e, as 16 K=8 matmuls on 32-aligned
            # row groups (psum recycled from gating tag). Placed before the
            # expert stream so the fp8 DoubleRow matmuls run uninterrupted.
            for nt in range(NT):
                g, r = nt // 4, 32 * (nt % 4)
                for boh in range(OH):
                    psum_b = gp.tile([P, FO], F32, tag="g", name="psum_b")
                    nc.tensor.matmul(
                        psum_b,
                        gatesT[g][r : r + E, :],
                        be_sb[r : r + E, boh * FO : (boh + 1) * FO],
                        start=True,
                        stop=True,
                        tile_position=(r, 0),
                    )
                    nc.vector.tensor_add(
                        acc_sb[nt][boh], acc_sb[nt][boh], psum_b
                    )

            # ---- main loop: stream fp8 experts, accumulate delta-weighted
            # DoubleRow GEMMs. Drains: acc = (psum * delta) + acc ----
            for e in range(E):
                w_all = wpool.tile([P, DK2, 2, O], F8E4, tag="w", name=f"w{e}")
                nc.sync.dma_start(out=w_all, in_=wq[e])
                for oh in range(OH):
                    for nt in range(NT):
                        dcol = delta_at(nt)[:, e : e + 1]
                        acc = acc_sb[nt][oh]
                        if e == E - 1 and oh == OH - 1 and nt == NT - 1:
                            # final group: two 256-wide accumulation groups so
                            # the first half drains + DMAs while the second
                            # half's matmuls still stream -> shorter tail
                            for hh in range(2):
                                pool, tg = ((mmp, "mm"), (gp, "g"))[hh]
                                psum_h = pool.tile(
                                    [P, 256], F32, tag=tg, name="psum_fin"
                                )
                                lo = oh * FO + hh * 256
                                for dk in range(DK2):
                                    nc.tensor.matmul(
                                        psum_h,
                                        xq_sb[:, dk, :, nt * P : (nt + 1) * P],
                                        w_all[:, dk, :, lo : lo + 256],
                                        start=(dk == 0),
                                        stop=(dk == DK2 - 1),
                                        perf_mode=DR,
                                    )
                                sl = slice(hh * 256, (hh + 1) * 256)
                                nc.vector.scalar_tensor_tensor(
                                    out=out_sb[nt][oh][:, sl],
                                    in0=psum_h,
                                    scalar=dcol,
                                    in1=acc[:, sl],
                                    op0=mybir.AluOpType.mult,
                                    op1=mybir.AluOpType.add,
                                )
                                oslice = out[nt, oh, :, sl]
                                if hh == 0:
                                    nc.sync.dma_start(
                                        out=oslice, in_=out_sb[nt][oh][:, sl]
                                    )
                                else:
                                    nc.scalar.dma_start(
                                        out=oslice, in_=out_sb[nt][oh][:, sl]
                                    )
                            continue
                        # alternate psum between both pools -> ~8 banks in
                        # flight, drains never gate the PE
                        if (oh * NT + nt) % 2 == 0:
                            psum = mmp.tile([P, FO], F32, tag="mm")
                        else:
                            psum = gp.tile([P, FO], F32, tag="g", name="psum_e")
                        for dk in range(DK2):
                            nc.tensor.matmul(
                                psum,
                                xq_sb[:, dk, :, nt * P : (nt + 1) * P],
                                w_all[:, dk, :, oh * FO : (oh + 1) * FO],
                                start=(dk == 0),
                                stop=(dk == DK2 - 1),
                                perf_mode=DR,
                            )
                        if e < E - 1:
                            nc.vector.scalar_tensor_tensor(
                                out=acc,
                                in0=psum,
                                scalar=dcol,
                                in1=acc,
                                op0=mybir.AluOpType.mult,
                                op1=mybir.AluOpType.add,
                            )
                        else:
                            # last expert: drain into bf16 out tiles in
                            # 256-wide chunks; DMAs alternate issue queues
                            for hh in range(2):
                                sl = slice(hh * 256, (hh + 1) * 256)
                                nc.vector.scalar_tensor_tensor(
                                    out=out_sb[nt][oh][:, sl],
                                    in0=psum[:, sl],
                                    scalar=dcol,
                                    in1=acc[:, sl],
                                    op0=mybir.AluOpType.mult,
                                    op1=mybir.AluOpType.add,
                                )
                                oslice = out[nt, oh, :, sl]
                                if (oh * NT + nt + hh) % 2 == 0:
                                    nc.sync.dma_start(
                                        out=oslice, in_=out_sb[nt][oh][:, sl]
                                    )
                                else:
                                    nc.scalar.dma_start(
                                        out=oslice, in_=out_sb[nt][oh][:, sl]
                                    )
    legalize_single_wait(nc)
    return nc


_NC_CACHE = {}


def _get_nc():
    if "nc" not in _NC_CACHE:
        _NC_CACHE["nc"] = build_moe()
    return _NC_CACHE["nc"]


def make_in_maps(x, W_e, b_e, W_g1, b_g1, W_g2, b_g2):
    x = np.asarray(x, dtype=np.float32)
    W_e = np.asarray(W_e, dtype=np.float32)
    # fp8 expert weights: [E, O, D] -> [E, P, DK2, 2, O] with logical
    # d = dk2*256 + i*128 + p
    wq = (
        (W_e * SW)
        .astype(F8)
        .transpose(0, 2, 1)               # [E, D, O]
        .reshape(E, DK2, 2, P, O)
        .transpose(0, 3, 1, 2, 4)         # [E, P, DK2, 2, O]
    )
    wq = np.ascontiguousarray(wq)
    # bf16 mean expert: [P, DK, O]
    wbar_f = W_e.mean(axis=0)             # [O, D]
    wbar = (
        wbar_f.T.reshape(DK, P, O).transpose(1, 0, 2)
    )
    wbar = np.ascontiguousarray(wbar).astype(BF)
    wg1t = (
        np.asarray(W_g1, dtype=np.float32).T.reshape(DK, P, H).transpose(1, 0, 2)
    )
    wg1t = np.ascontiguousarray(wg1t).astype(BF)
    wg2t = (
        np.asarray(W_g2, dtype=np.float32).T.reshape(H2, P, E).transpose(1, 0, 2)
    )
    wg2t = np.ascontiguousarray(wg2t).astype(BF)
    bg1 = np.ascontiguousarray(
        np.asarray(b_g1, dtype=np.float32).reshape(H2, P).T
    )
    bg2 = np.asarray(b_g2, dtype=np.float32).astype(BF)
    be_rep = np.zeros((P, O), dtype=np.float32)
    for g in range(4):
        be_rep[32 * g : 32 * g + E, :] = np.asarray(b_e, dtype=np.float32)
    be_rep = be_rep.astype(BF)
    ident_np = np.eye(P, dtype=np.float32)
    xb = x.astype(BF)
    xq8 = (x * SX).astype(F8)
    in_maps = []
    for c in range(NCORES):
        xrow = xb[c * NLOC : (c + 1) * NLOC, :]
        xT_c = (
            np.asarray(xrow.T).reshape(DK, P, NLOC).transpose(1, 0, 2)
        )
        xT_c = np.ascontiguousarray(xT_c)
        xq_c = (
            np.asarray(xq8[c * NLOC : (c + 1) * NLOC, :].T)
            .reshape(DK2, 2, P, NLOC)
            .transpose(2, 0, 1, 3)        # [P, DK2, 2, NLOC]
        )
        xq_c = np.ascontiguousarray(xq_c)
        in_maps.append(
            {
                "xT": xT_c,
                "xq": xq_c,
                "wbar": wbar,
                "wq": wq,
                "wg1t": wg1t,
                "wg2t": wg2t,
                "bg1": bg1,
                "bg2": bg2,
                "be_rep": be_rep,
                "ident": ident_np,
            }
        )
    return in_maps


def kernel(x, W_e, b_e, W_g1, b_g1, W_g2, b_g2, **run_kwargs):
    nc = _get_nc()
    in_maps = make_in_maps(x, W_e, b_e, W_g1, b_g1, W_g2, b_g2)
    res = run_bass_kernel_spmd(nc, in_maps, core_ids=list(range(NCORES)), **run_kwargs)
    outs = []
    for c in range(NCORES):
        o = np.asarray(res.results[c]["out"])  # [NT, OH, P, FO] bf16
        outs.append(
            o.astype(np.float32).transpose(0, 2, 1, 3).reshape(NLOC, O)
        )
    out = np.concatenate(outs, axis=0)
    if run_kwargs:
        kernel.last_results = res
    return out


if __name__ == "__main__":
    rng = np.random.default_rng(0)
    s = 1.0 / np.sqrt(D)
    sh = 1.0 / np.sqrt(H)
    inputs = {
        "x": rng.standard_normal((N, D), dtype=np.float32),
        "W_e": rng.uniform(-s, s, (E, O, D)).astype(np.float32),
        "b_e": rng.uniform(-s, s, (E, O)).astype(np.float32),
        "W_g1": rng.uniform(-s, s, (H, D)).astype(np.float32),
        "b_g1": rng.uniform(-sh, sh, (H,)).astype(np.float32),
        "W_g2": rng.uniform(-sh, sh, (E, H)).astype(np.float32),
        "b_g2": rng.uniform(-sh, sh, (E,)).astype(np.float32),
    }
    out = kernel(**inputs)
    print("out", out.shape, out.dtype, float(np.abs(out).max()))


# revision 17
# speedup vs baseline: 1.0175x; 1.0175x over previous
"""Dense MoE (all-experts, gate-weighted sum) on 8 Trainium2 NeuronCores.

Sharding: pure data-parallel over the token axis N (8192 -> 1024 rows/core);
every core holds all 8 experts, so no collectives are needed.

Mean-centered fp8 decomposition (the key trick):
    out = x @ Wbar.T  +  sum_e (g_e - 1/8) * (x @ W_e.T)  +  gates @ b_e
with Wbar = mean_e W_e. The bulk term x@Wbar runs in bf16; the 8 expert
GEMMs run as e4m3 DoubleRow matmuls (K=256/instruction -> 2x PE through-
put, measured 216 ns per FD=512 DR matmul, same as bf16). Because the
correction weights delta_e = g_e - 1/8 have std ~0.03, the fp8
quantization noise of the expert GEMMs is attenuated ~30x in the output:
simulated end-to-end rel err ~1.1e-2 (pure-fp8 would be 3.3e-2).

Per core (N_loc=1024, D=1024, E=8, O=1024, H=256) the PE busy time is
    gating 6.9us + Wbar(bf16) 27.6us + experts(fp8 DR) 110.6us + misc
vs 218us+ for all-bf16 experts.

Schedule skeleton is inherited from the bf16 baseline: host-prearranged
layouts (partition-major, large DMA descriptors); warmup+filler matmuls
hold the PE HAM clock at 2.4 GHz across the initial DMA; the gating GEMM
and seven Wbar "wave-1" PSUM groups accumulate dk-progressively as x/Wbar
quads land on the sync ring; logits for all 8 n-tiles collect in one PSUM
bank -> single batched softmax (exp preloaded, no max-subtract); gates.T
via 2 zero-padded PE transposes feeds 16 K=8 bias matmuls; expert drains
are one fused DVE op acc = (psum * delta) + acc. The dequant scale
1/(SX*SW) is folded into delta on device. The last expert drains into
bf16 tiles (host upcasts to fp32), halving output DMA bytes; its final
tile drains in 256-wide chunks with DMAs alternating across issue queues.
"""

import numpy as np
import ml_dtypes

import concourse.bass as bass
import concourse.mybir as mybir
import concourse.tile as tile
from concourse.bass_utils import run_bass_kernel_spmd

N, D, E, O, H = 8192, 1024, 8, 1024, 256
NCORES = 8
NLOC = N // NCORES          # 1024 rows per core
P = 128                     # partitions
NT = NLOC // P              # 8 n-tiles
DK = D // P                 # 8 contraction tiles (bf16 path)
DK2 = D // 256              # 4 double-row contraction tiles (fp8 path)
FO = 512                    # matmul moving free dim (one PSUM bank of fp32)
OH = O // FO                # 2 output halves
H2 = H // P                 # 2 h-tiles
BF16 = mybir.dt.bfloat16
F8E4 = mybir.dt.float8e4
F32 = mybir.dt.float32
BF = ml_dtypes.bfloat16
F8 = ml_dtypes.float8_e4m3fn
DR = mybir.MatmulPerfMode.DoubleRow
N_WARM = 8                  # pre-stream HAM warmup matmuls
N_FILL = 16                 # filler matmuls bridging the DMA-paced phase
W1A = 4                     # wave-1a: Wbar (oh0, nt<4) groups in mm pool
W1B = 3                     # wave-1b: Wbar (oh1, nt<3) groups in g pool
SX = 32.0                   # host scale for x -> e4m3
SW = 2048.0                 # host scale for W_e -> e4m3
DEQ = 1.0 / (SX * SW)       # folded into delta on device


def legalize_single_wait(nc, max_waits=1):
    """This walrus build rejects instructions carrying more than one sync
    wait. Split each multi-wait instruction: excess waits move onto fresh
    same-engine NoOps inserted immediately before it (identical semantics:
    the engine stalls at the same program point on every semaphore)."""
    for f in nc.m.functions:
        for blk in f.blocks:
            insts = list(blk.instructions)
            if all(
                (i.sync_info is None or len(i.sync_info.on_wait) <= max_waits)
                for i in insts
            ):
                continue
            new = []
            for inst in insts:
                si = inst.sync_info
                if si is not None and len(si.on_wait) > max_waits:
                    waits = list(si.on_wait)
                    for k, w in enumerate(waits[:-max_waits]):
                        nop = mybir.InstNoOp(name=f"{inst.name}-w{k}")
                        nop.engine = inst.engine
                        nop.sync_info = mybir.SyncInfo(on_wait=[w], on_update=[])
                        new.append(nop)
                    si.on_wait = waits[-max_waits:]
                new.append(inst)
            blk.instructions = new
    return nc


def build_moe():
    nc = bass.Bass(target_bir_lowering=False)
    xT = nc.dram_tensor("xT", [P, DK, NLOC], BF16, kind="ExternalInput")
    xq = nc.dram_tensor("xq", [P, DK2, 2, NLOC], F8E4, kind="ExternalInput")
    wbar = nc.dram_tensor("wbar", [P, DK, O], BF16, kind="ExternalInput")
    wq = nc.dram_tensor("wq", [E, P, DK2, 2, O], F8E4, kind="ExternalInput")
    wg1t = nc.dram_tensor("wg1t", [P, DK, H], BF16, kind="ExternalInput")
    wg2t = nc.dram_tensor("wg2t", [P, H2, E], BF16, kind="ExternalInput")
    bg1 = nc.dram_tensor("bg1", [P, H2], F32, kind="ExternalInput")
    bg2 = nc.dram_tensor("bg2", [E], BF16, kind="ExternalInput")
    be_rep = nc.dram_tensor("be_rep", [P, 4, O], BF16, kind="ExternalInput")
    ident = nc.dram_tensor("ident", [P, P], F32, kind="ExternalInput")
    out = nc.dram_tensor("out", [NT, OH, P, FO], BF16, kind="ExternalOutput")

    with tile.TileContext(nc) as tc:
        with (
            tc.tile_pool(name="const", bufs=1) as constp,
            tc.tile_pool(name="wpool", bufs=2) as wpool,
            tc.tile_pool(name="work", bufs=4) as workp,
            tc.tile_pool(name="g_ps", bufs=4, space="PSUM") as gp,
            tc.tile_pool(name="mm_ps", bufs=4, space="PSUM") as mmp,
        ):
            # ---- PE warm-up + ACT Exp-table preload during the DMA wait ----
            warm_a = constp.tile([P, P], BF16, tag="warm_a")
            nc.gpsimd.memset(warm_a, 0.0)
            warm_b = constp.tile([P, FO], BF16, tag="warm_b")
            nc.gpsimd.memset(warm_b, 0.0)
            for i in range(N_WARM):
                wpsum = mmp.tile([P, FO], F32, tag="mm", name=f"warm{i}")
                nc.tensor.matmul(wpsum, warm_a, warm_b, start=True, stop=True)
            dummy_exp = workp.tile([1, 1], F32, tag="dummy")
            nc.scalar.activation(
                out=dummy_exp,
                in_=warm_b[0:1, 0:1],
                func=mybir.ActivationFunctionType.Exp,
            )

            # ---- resident inputs. sync ring (FIFO = priority): x halves
            # interleaved with Wbar halves, then xq, then fp8 experts.
            # gpsimd ring: gating weights + small consts ----
            wg1t_sb = constp.tile([P, DK, H], BF16, tag="wg1t")
            xT_sb = constp.tile([P, DK, NLOC], BF16, tag="xT")
            wbar_sb = constp.tile([P, DK, O], BF16, tag="wbar")
            xq_sb = constp.tile([P, DK2, 2, NLOC], F8E4, tag="xq")
            nc.gpsimd.dma_start(out=wg1t_sb, in_=wg1t[:, :, :])
            # first x chunks split fine so gating matmuls start asap
            nc.sync.dma_start(out=xT_sb[:, 0:1, :], in_=xT[:, 0:1, :])
            nc.sync.dma_start(out=xT_sb[:, 1:2, :], in_=xT[:, 1:2, :])
            nc.sync.dma_start(out=xT_sb[:, 2:4, :], in_=xT[:, 2:4, :])
            nc.sync.dma_start(out=wbar_sb[:, 0:4, :], in_=wbar[:, 0:4, :])
            nc.sync.dma_start(out=xT_sb[:, 4:8, :], in_=xT[:, 4:8, :])
            nc.sync.dma_start(out=wbar_sb[:, 4:8, :], in_=wbar[:, 4:8, :])
            nc.sync.dma_start(out=xq_sb, in_=xq[:, :, :, :])

            wg2t_sb = constp.tile([P, H2, E], BF16, tag="wg2t")
            nc.gpsimd.dma_start(out=wg2t_sb, in_=wg2t[:, :, :])
            bg1_sb = constp.tile([P, H2], F32, tag="bg1")
            nc.gpsimd.dma_start(out=bg1_sb, in_=bg1[:, :])
            bg2_sb = constp.tile([1, E], BF16, tag="bg2")
            nc.gpsimd.dma_start(out=bg2_sb, in_=bg2[:])
            ones_sb = constp.tile([1, P], BF16, tag="ones")
            nc.gpsimd.memset(ones_sb, 1.0)
            # ident/be_rep are needed only from bias-matmul time: they ride
            # the sync ring AFTER xq (FIFO ring = priority)
            ident_sb = constp.tile([P, P], F32, tag="ident")
            nc.sync.dma_start(out=ident_sb, in_=ident[:, :])
            be_sb = constp.tile([P, 4, O], BF16, tag="be_rep")
            nc.sync.dma_start(out=be_sb, in_=be_rep[:, :, :])

            # ---- gating GEMM (4 psum banks) + wave-1 Wbar groups,
            # accumulating dk-progressively as halves land ----
            hT_sb = [
                constp.tile([P, NLOC], BF16, tag=f"hT{h2}", name=f"hT{h2}")
                for h2 in range(H2)
            ]
            psum_g = {
                (h2, nh): gp.tile([P, FO], F32, tag="g", name=f"psum_g{h2}_{nh}")
                for h2 in range(H2)
                for nh in range(NLOC // FO)
            }

            def gating_mms(dks):
                for dk in dks:
                    for h2 in range(H2):
                        for nh in range(NLOC // FO):
                            nc.tensor.matmul(
                                psum_g[(h2, nh)],
                                wg1t_sb[:, dk, h2 * P : (h2 + 1) * P],
                                xT_sb[:, dk, nh * FO : (nh + 1) * FO],
                                start=(dk == 0),
                                stop=(dk == DK - 1),
                            )

            def gating_one(dk):
                gating_mms(range(dk, dk + 1))

            # fillers: bridge the PE from the warmup to the first xT quad
            # landing (DMA-bound) so the HAM clock never re-throttles
            for i in range(N_FILL):
                wpsum = mmp.tile([P, FO], F32, tag="mm", name=f"fill{i}")
                nc.tensor.matmul(wpsum, warm_a, warm_b, start=True, stop=True)

            gating_one(0)
            gating_one(1)
            gating_mms(range(2, 4))

            # wave-1a: four Wbar (oh0) groups accumulate dk 0-3 as soon as
            # the first wbar quad lands (mm-pool banks)
            psum_w1a = [
                mmp.tile([P, FO], F32, tag="mm", name=f"w1a{i}") for i in range(W1A)
            ]

            def wave1a_mms(dks):
                for i in range(W1A):         # (nt=i, oh=0)
                    for dk in dks:
                        nc.tensor.matmul(
                            psum_w1a[i],
                            xT_sb[:, dk, i * P : (i + 1) * P],
                            wbar_sb[:, dk, 0:FO],
                            start=(dk == 0),
                            stop=(dk == DK - 1),
                        )

            wave1a_mms(range(0, 4))
            gating_mms(range(4, 8))

            # relus on DVE, ordered so logits for nt<4 unblock after 2 ops
            for nh in range(NLOC // FO):
                for h2 in range(H2):
                    nc.vector.tensor_scalar(
                        out=hT_sb[h2][:, nh * FO : (nh + 1) * FO],
                        in0=psum_g[(h2, nh)],
                        scalar1=bg1_sb[:, h2 : h2 + 1],
                        scalar2=0.0,
                        op0=mybir.AluOpType.add,
                        op1=mybir.AluOpType.max,
                    )

            # wave-1b + logits recycle the gating pool's banks (freed by the
            # relus above)
            psum_w1b = [
                gp.tile([P, FO], F32, tag="g", name=f"w1b{i}") for i in range(W1B)
            ]
            psum_l = gp.tile([P, NT, E], F32, tag="g", name="psum_l")

            def wave1b_mms(dks):
                for i in range(W1B):         # (nt=i, oh=1)
                    for dk in dks:
                        nc.tensor.matmul(
                            psum_w1b[i],
                            xT_sb[:, dk, i * P : (i + 1) * P],
                            wbar_sb[:, dk, FO : 2 * FO],
                            start=(dk == 0),
                            stop=(dk == DK - 1),
                        )

            wave1b_mms(range(0, 4))

            # logits for all n-tiles in ONE psum bank
            for nt in range(NT):
                for h2 in range(H2):
                    nc.tensor.matmul(
                        psum_l[:, nt, :],
                        hT_sb[h2][:, nt * P : (nt + 1) * P],
                        wg2t_sb[:, h2, :],
                        start=(h2 == 0),
                        stop=False,
                    )
                nc.tensor.matmul(
                    psum_l[:, nt, :], ones_sb, bg2_sb, start=False, stop=True
                )

            wave1a_mms(range(4, 8))
            wave1b_mms(range(4, 8))

            # batched softmax (no max-subtract: logits are O(1)); gates
            # zero-padded to 32 per n-tile so the transposed layout is
            # 32-row aligned (tile_position row groups for the bias matmuls)
            EP = 32
            gates_g = [
                constp.tile([P, NT // 2, EP], F32, tag=f"gates{g}", name=f"gates{g}")
                for g in range(2)
            ]
            for g in range(2):
                nc.vector.memset(gates_g[g], 0.0)
            gates_at = lambda nt: gates_g[nt // 4][:, nt % 4, 0:E]
            for g in range(2):
                nc.scalar.activation(
                    out=gates_g[g][:, :, 0:E],
                    in_=psum_l[:, 4 * g : 4 * (g + 1), :],
                    func=mybir.ActivationFunctionType.Exp,
                )
            sumexp = workp.tile([P, NT, 1], F32, tag="sumexp")
            for g in range(2):
                nc.vector.reduce_sum(
                    sumexp[:, 4 * g : 4 * (g + 1), :],
                    gates_g[g][:, :, 0:E],
                    axis=mybir.AxisListType.X,
                )
            rsum = workp.tile([P, NT, 1], F32, tag="rsum")
            nc.vector.reciprocal(rsum, sumexp)
            for nt in range(NT):
                nc.vector.tensor_scalar_mul(
                    gates_at(nt), gates_at(nt), rsum[:, nt, :]
                )

            # delta = (gates - 1/8) * DEQ  (dequant scale folded in);
            # padding columns get garbage but are never read
            delta_g = [
                constp.tile([P, NT // 2, EP], F32, tag=f"delta{g}", name=f"delta{g}")
                for g in range(2)
            ]
            for g in range(2):
                nc.vector.tensor_scalar(
                    out=delta_g[g],
                    in0=gates_g[g],
                    scalar1=-1.0 / E,
                    scalar2=DEQ,
                    op0=mybir.AluOpType.add,
                    op1=mybir.AluOpType.mult,
                )
            delta_at = lambda nt: delta_g[nt // 4][:, nt % 4, 0:E]

            acc_sb = [
                [
                    constp.tile(
                        [P, FO], F32, tag=f"acc{nt}_{oh}", name=f"acc{nt}_{oh}"
                    )
                    for oh in range(OH)
                ]
                for nt in range(NT)
            ]
            # bf16 staging tiles for the output of the last expert
            out_sb = [
                [
                    constp.tile(
                        [P, FO], BF16, tag=f"out{nt}_{oh}", name=f"out{nt}_{oh}"
                    )
                    for oh in range(OH)
                ]
                for nt in range(NT)
            ]

            # wave-1 drains: acc = psum (the Wbar term initializes acc);
            # alternate ACT/DVE to halve the drain backlog
            for i in range(W1A):
                if i % 2 == 0:
                    nc.scalar.copy(out=acc_sb[i][0], in_=psum_w1a[i])
                else:
                    nc.vector.tensor_copy(acc_sb[i][0], psum_w1a[i])
            for i in range(W1B):
                if i % 2 == 0:
                    nc.vector.tensor_copy(acc_sb[i][1], psum_w1b[i])
                else:
                    nc.scalar.copy(out=acc_sb[i][1], in_=psum_w1b[i])

            # gates.T via two 128x128 PE transposes (rows nt*32+e, 32-aligned),
            # hidden inside the Wbar matmul stream
            gatesT = []
            for g in range(2):
                psum_t = gp.tile([P, P], F32, tag="g", name="psum_t")
                nc.tensor.transpose(
                    psum_t, gates_g[g].rearrange("p a b -> p (a b)"), ident_sb
                )
                gT = constp.tile([P, P], BF16, tag=f"gatesT{g}", name=f"gatesT{g}")
                nc.scalar.copy(out=gT, in_=psum_t)
                gatesT.append(gT)

            # ---- remaining 9 Wbar tiles (closed groups, mm pool) ----
            def wbar_tile(nt, oh, pool, tg, j):
                psum = pool.tile([P, FO], F32, tag=tg, name=f"wb{nt}_{oh}")
                for dk in range(DK):
                    nc.tensor.matmul(
                        psum,
                        xT_sb[:, dk, nt * P : (nt + 1) * P],
                        wbar_sb[:, dk, oh * FO : (oh + 1) * FO],
                        start=(dk == 0),
                        stop=(dk == DK - 1),
                    )
                # alternate drains across ACT and DVE so the drain backlog
                # clears ~2x faster and never stalls the expert stream
                if j % 2 == 0:
                    nc.scalar.copy(out=acc_sb[nt][oh], in_=psum)
                else:
                    nc.vector.tensor_copy(acc_sb[nt][oh], psum)

            rest = [(nt, 0) for nt in range(W1A, NT)] + [
                (nt, 1) for nt in range(W1B, NT)
            ]
            for j, (nt, oh) in enumerate(rest):
                pool, tg = ((mmp, "mm"), (gp, "g"))[j % 2]
                wbar_tile(nt, oh, pool, tg, j)

            # bias term: out += gates @ b_e, as 16 full-K matmuls against a
            # zero-padded per-strip bias (be_sb[:, nt%4]: rows 32a+e hold
            # b_e, others zero -> picks out exactly n-tile nt's strip of
            # gatesT). Placed before the expert stream so the fp8 DoubleRow
            # matmuls run uninterrupted.
            for nt in range(NT):
                g, a = nt // 4, nt % 4
                for boh in range(OH):
                    psum_b = gp.tile([P, FO], F32, tag="g", name="psum_b")
                    nc.tensor.matmul(
                        psum_b,
                        gatesT[g],
                        be_sb[:, a, boh * FO : (boh + 1) * FO],
                        start=True,
                        stop=True,
                    )
                    nc.vector.tensor_add(
                        acc_sb[nt][boh], acc_sb[nt][boh], psum_b
                    )

            # ---- main loop: stream fp8 experts, accumulate delta-weighted
            # DoubleRow GEMMs. Drains: acc = (psum * delta) + acc ----
            for e in range(E):
                w_all = wpool.tile([P, DK2, 2, O], F8E4, tag="w", name=f"w{e}")
                nc.sync.dma_start(out=w_all, in_=wq[e])
                for oh in range(OH):
                    for nt in range(NT):
                        dcol = delta_at(nt)[:, e : e + 1]
                        acc = acc_sb[nt][oh]
                        if e == E - 1 and oh == OH - 1 and nt == NT - 1:
                            # final group: two 256-wide accumulation groups so
                            # the first half drains + DMAs while the second
                            # half's matmuls still stream -> shorter tail
                            for hh in range(2):
                                pool, tg = ((mmp, "mm"), (gp, "g"))[hh]
                                psum_h = pool.tile(
                                    [P, 256], F32, tag=tg, name="psum_fin"
                                )
                                lo = oh * FO + hh * 256
                                for dk in range(DK2):
                                    nc.tensor.matmul(
                                        psum_h,
                                        xq_sb[:, dk, :, nt * P : (nt + 1) * P],
                                        w_all[:, dk, :, lo : lo + 256],
                                        start=(dk == 0),
                                        stop=(dk == DK2 - 1),
                                        perf_mode=DR,
                                    )
                                sl = slice(hh * 256, (hh + 1) * 256)
                                nc.vector.scalar_tensor_tensor(
                                    out=out_sb[nt][oh][:, sl],
                                    in0=psum_h,
                                    scalar=dcol,
                                    in1=acc[:, sl],
                                    op0=mybir.AluOpType.mult,
                                    op1=mybir.AluOpType.add,
                                )
                                oslice = out[nt, oh, :, sl]
                                if hh == 0:
                                    nc.sync.dma_start(
                                        out=oslice, in_=out_sb[nt][oh][:, sl]
                                    )
                                else:
                                    nc.scalar.dma_start(
                                        out=oslice, in_=out_sb[nt][oh][:, sl]
                                    )
                            continue
                        # alternate psum between both pools -> ~8 banks in
                        # flight, drains never gate the PE
                        if (oh * NT + nt) % 2 == 0:
                            psum = mmp.tile([P, FO], F32, tag="mm")
                        else:
                            psum = gp.tile([P, FO], F32, tag="g", name="psum_e")
                        for dk in range(DK2):
                            nc.tensor.matmul(
                                psum,
                                xq_sb[:, dk, :, nt * P : (nt + 1) * P],
                                w_all[:, dk, :, oh * FO : (oh + 1) * FO],
                                start=(dk == 0),
                                stop=(dk == DK2 - 1),
                                perf_mode=DR,
                            )
                        if e < E - 1:
                            nc.vector.scalar_tensor_tensor(
                                out=acc,
                                in0=psum,
                                scalar=dcol,
                                in1=acc,
                                op0=mybir.AluOpType.mult,
                                op1=mybir.AluOpType.add,
                            )
                        else:
                            # last expert: drain into bf16 out tiles in
                            # 256-wide chunks; DMAs alternate issue queues
                            for hh in range(2):
                                sl = slice(hh * 256, (hh + 1) * 256)
                                nc.vector.scalar_tensor_tensor(
                                    out=out_sb[nt][oh][:, sl],
                                    in0=psum[:, sl],
                                    scalar=dcol,
                                    in1=acc[:, sl],
                                    op0=mybir.AluOpType.mult,
                                    op1=mybir.AluOpType.add,
                                )
                                oslice = out[nt, oh, :, sl]
                                if (oh * NT + nt + hh) % 2 == 0:
                                    nc.sync.dma_start(
                                        out=oslice, in_=out_sb[nt][oh][:, sl]
                                    )
                                else:
                                    nc.scalar.dma_start(
                                        out=oslice, in_=out_sb[nt][oh][:, sl]
                                    )
    legalize_single_wait(nc)
    return nc


_NC_CACHE = {}


def _get_nc():
    if "nc" not in _NC_CACHE:
        _NC_CACHE["nc"] = build_moe()
    return _NC_CACHE["nc"]


def make_in_maps(x, W_e, b_e, W_g1, b_g1, W_g2, b_g2):
    x = np.asarray(x, dtype=np.float32)
    W_e = np.asarray(W_e, dtype=np.float32)
    # fp8 expert weights: [E, O, D] -> [E, P, DK2, 2, O] with logical
    # d = dk2*256 + i*128 + p
    wq = (
        (W_e * SW)
        .astype(F8)
        .transpose(0, 2, 1)               # [E, D, O]
        .reshape(E, DK2, 2, P, O)
        .transpose(0, 3, 1, 2, 4)         # [E, P, DK2, 2, O]
    )
    wq = np.ascontiguousarray(wq)
    # bf16 mean expert: [P, DK, O]
    wbar_f = W_e.mean(axis=0)             # [O, D]
    wbar = (
        wbar_f.T.reshape(DK, P, O).transpose(1, 0, 2)
    )
    wbar = np.ascontiguousarray(wbar).astype(BF)
    wg1t = (
        np.asarray(W_g1, dtype=np.float32).T.reshape(DK, P, H).transpose(1, 0, 2)
    )
    wg1t = np.ascontiguousarray(wg1t).astype(BF)
    wg2t = (
        np.asarray(W_g2, dtype=np.float32).T.reshape(H2, P, E).transpose(1, 0, 2)
    )
    wg2t = np.ascontiguousarray(wg2t).astype(BF)
    bg1 = np.ascontiguousarray(
        np.asarray(b_g1, dtype=np.float32).reshape(H2, P).T
    )
    bg2 = np.asarray(b_g2, dtype=np.float32).astype(BF)
    # zero-padded per-strip bias: be_rep[32a+e, a, :] = b_e (so a full-K
    # matmul against gatesT picks out exactly one n-tile's strip)
    be_rep = np.zeros((P, 4, O), dtype=np.float32)
    for a in range(4):
        be_rep[32 * a : 32 * a + E, a, :] = np.asarray(b_e, dtype=np.float32)
    be_rep = be_rep.astype(BF)
    ident_np = np.eye(P, dtype=np.float32)
    xb = x.astype(BF)
    xq8 = (x * SX).astype(F8)
    in_maps = []
    for c in range(NCORES):
        xrow = xb[c * NLOC : (c + 1) * NLOC, :]
        xT_c = (
            np.asarray(xrow.T).reshape(DK, P, NLOC).transpose(1, 0, 2)
        )
        xT_c = np.ascontiguousarray(xT_c)
        xq_c = (
            np.asarray(xq8[c * NLOC : (c + 1) * NLOC, :].T)
            .reshape(DK2, 2, P, NLOC)
            .transpose(2, 0, 1, 3)        # [P, DK2, 2, NLOC]
        )
        xq_c = np.ascontiguousarray(xq_c)
        in_maps.append(
            {
                "xT": xT_c,
                "xq": xq_c,
                "wbar": wbar,
                "wq": wq,
                "wg1t": wg1t,
                "wg2t": wg2t,
                "bg1": bg1,
                "bg2": bg2,
                "be_rep": be_rep,
                "ident": ident_np,
            }
        )
    return in_maps


def kernel(x, W_e, b_e, W_g1, b_g1, W_g2, b_g2, **run_kwargs):
    nc = _get_nc()
    in_maps = make_in_maps(x, W_e, b_e, W_g1, b_g1, W_g2, b_g2)
    res = run_bass_kernel_spmd(nc, in_maps, core_ids=list(range(NCORES)), **run_kwargs)
    outs = []
    for c in range(NCORES):
        o = np.asarray(res.results[c]["out"])  # [NT, OH, P, FO] bf16
        outs.append(
            o.astype(np.float32).transpose(0, 2, 1, 3).reshape(NLOC, O)
        )
    out = np.concatenate(outs, axis=0)
    if run_kwargs:
        kernel.last_results = res
    return out


if __name__ == "__main__":
    rng = np.random.default_rng(0)
    s = 1.0 / np.sqrt(D)
    sh = 1.0 / np.sqrt(H)
    inputs = {
        "x": rng.standard_normal((N, D), dtype=np.float32),
        "W_e": rng.uniform(-s, s, (E, O, D)).astype(np.float32),
        "b_e": rng.uniform(-s, s, (E, O)).astype(np.float32),
        "W_g1": rng.uniform(-s, s, (H, D)).astype(np.float32),
        "b_g1": rng.uniform(-sh, sh, (H,)).astype(np.float32),
        "W_g2": rng.uniform(-sh, sh, (E, H)).astype(np.float32),
        "b_g2": rng.uniform(-sh, sh, (E,)).astype(np.float32),
    }
    out = kernel(**inputs)
    print("out", out.shape, out.dtype, float(np.abs(out).max()))
